# revision 21
# baseline (speedup 1.0000x reference)
"""GQA kernel for trn2, 8 NeuronCores — bf16 version.

Problem: B=2, N=2048, d_model=2048, 32 q heads / 8 kv heads, d_head=64.
Sharding: batch (2) x head-groups (4): core c = b*4 + g handles batch b and
q heads [8g, 8g+8) (kv heads [2g, 2g+1]).  Each core computes
partial_out = attn_out_g @ Wo[:, cols_g].T ; host sums the 4 group partials
per batch and adds bo.

Key differences vs the fp32 baseline:
  * all matmul operands bf16 (1 cyc/row on PE vs 4 for fp32)
  * x is transposed on the host (xT input) — no PE transposes
  * softmax denominator fused into the AV matmul via a ones column in the
    65-wide V' stationary (no separate ones-matmul per key chunk)
  * exp over [128,1024] PSUM (both head halves at once) to halve ACT
    instruction count
  * out-projection of q-tile qt-1 interleaved into attention of qt to
    fill PE bubbles in the ACT-bound inner loop

Per-core layouts (bf16 unless noted):
  xT_sb [128, 16*2048]  chunk ck = xT rows [128ck,128ck+128), free = tokens
  qT  [128, 4*2048]  chunk j holds heads (j, j+4): partitions 0:64 = head j
                     dims, 64:128 = head j+4 dims; free = tokens.
  kT  [128, 2048]    partitions 0:64 = kv0 k-dims, 64:128 = kv1 k-dims.
  vp0/vp1 [128, 16*65]  chunk kc = [64 v-dims | ones]: AV out rows 0:64 =
                     attn numerator, row 64 = denominator.  Half-1 data is
                     partition-shifted 0:64 -> 64:128 by a PSUM->SBUF DMA
                     before the normalize multiply (PE out base partitions
                     must be 0/32/64, so it can't land at 63:128 directly).
"""

import numpy as np
import ml_dtypes

import concourse.bass as bass
import concourse.mybir as mybir
from concourse.tile import TileContext, add_dep_helper
from concourse.bass_utils import run_bass_kernel_spmd


def _split_matmul_waits(bir_bytes):
    """Walrus in this toolchain allows only ONE sync wait per Matmult.

    For any matmul carrying N>1 waits, insert a PE NoOp immediately
    before it holding the first N-1 waits; the matmul keeps the last.
    The NoOp precedes the matmul in the PE stream, so ordering
    semantics are identical.
    """
    import json as _json
    bir = _json.loads(bir_bytes)
    n = 0
    for f in bir["functions"]:
        for b in f["blocks"]:
            out = []
            for i in b["instructions"]:
                si = i.get("sync_info") if isinstance(i, dict) else None
                eng = i.get("engine") if isinstance(i, dict) else None
                if (si and len(si.get("on_wait", [])) > 1
                        and eng and eng != "Unassigned"):
                    waits = si["on_wait"]
                    for w in waits[:-1]:
                        out.append({
                            "debug": i.get("debug", 0),
                            "engine": eng,
                            "ins": [], "outs": [],
                            "name": "%s-w%d" % (i["name"], n),
                            "opcode": "NoOp",
                            "sync_info": {"on_update": [], "on_wait": [w]},
                        })
                        n += 1
                    si["on_wait"] = waits[-1:]
                out.append(i)
            b["instructions"] = out
    return _json.dumps(bir).encode()


def _pe_touch(nc, producers):
    """Advance PE's vector clock past each producer, one sem at a time."""
    for p in producers:
        n = nc.tensor.nop()
        add_dep_helper(n.ins, p.ins, sync=True, reason="pe-wait-absorber")


F32 = mybir.dt.float32
BF16 = mybir.dt.bfloat16
AF = mybir.ActivationFunctionType

D = 2048      # d_model
TOKS = 2048   # tokens per batch
QD = 512      # q dims per core
DH = 64
NCK = 16      # d_model chunks of 128
TT = 512      # token tile for projections
NTT = TOKS // TT
QTILE = 512
NQT = TOKS // QTILE
NKC = TOKS // 128   # key chunks of 128
V65 = DH + 1        # V' stationary width incl. ones column
SCALE = DH ** -0.5  # 0.125

# local head order within a core: chunk j holds heads (j, j+4)
HEAD_ORDER = [0, 4, 1, 5, 2, 6, 3, 7]


def _build():
    nc = bass.Bass()
    xT = nc.declare_dram_parameter("xT", [D, TOKS], BF16, isOutput=False)
    wqT = nc.declare_dram_parameter("wqT", [D, QD], BF16, isOutput=False)
    wkT = nc.declare_dram_parameter("wkT", [D, 128], BF16, isOutput=False)
    wvT = nc.declare_dram_parameter("wvT", [D, 128], BF16, isOutput=False)
    woT = nc.declare_dram_parameter("woT", [QD, D], BF16, isOutput=False)
    bq4 = nc.declare_dram_parameter("bq4", [128, 4], F32, isOutput=False)
    bkT = nc.declare_dram_parameter("bkT", [128, 1], F32, isOutput=False)
    bvT = nc.declare_dram_parameter("bvT", [128, 1], F32, isOutput=False)
    eye = nc.declare_dram_parameter("eye", [128, 128], BF16, isOutput=False)
    out = nc.declare_dram_parameter("out", [TOKS, D], F32, isOutput=True)

    with TileContext(nc) as tc, \
            nc.allow_low_precision(reason="bf16 kernel; tol 2e-2"), \
            tc.tile_pool(name="persist", bufs=1) as pp:
        if True:
            xT_sb = pp.tile([128, NCK * TOKS], BF16, tag="xT")
            wq_sb = pp.tile([128, NCK * QD], BF16, tag="wq")
            wk_sb = pp.tile([128, NCK * 128], BF16, tag="wk")
            wv_sb = pp.tile([128, NCK * 128], BF16, tag="wv")
            wo_sb = pp.tile([128, 4 * D], BF16, tag="wo")
            qT = pp.tile([128, 4 * TOKS], BF16, tag="qT")
            kT = pp.tile([128, TOKS], BF16, tag="kT")
            vT_sb = pp.tile([128, TOKS], BF16, tag="vT")
            vp0 = pp.tile([128, NKC * V65], BF16, tag="vp0")
            vp1 = pp.tile([128, NKC * V65], BF16, tag="vp1")
            ones65 = pp.tile([65, DH], BF16, tag="ones65")
            eye_sb = pp.tile([128, 128], BF16, tag="eye")
            bq_sb = pp.tile([128, 4], F32, tag="bq")
            bk_sb = pp.tile([128, 1], F32, tag="bk")
            bv_sb = pp.tile([128, 1], F32, tag="bv")

            const_loads = []
            for ck in range(NCK):
                const_loads.append(nc.sync.dma_start(
                    out=wk_sb[:, ck * 128:(ck + 1) * 128],
                    in_=wkT[ck * 128:(ck + 1) * 128, :]))
                const_loads.append(nc.sync.dma_start(
                    out=xT_sb[:, ck * TOKS:(ck + 1) * TOKS],
                    in_=xT[ck * 128:(ck + 1) * 128, :]))
                const_loads.append(nc.sync.dma_start(
                    out=wv_sb[:, ck * 128:(ck + 1) * 128],
                    in_=wvT[ck * 128:(ck + 1) * 128, :]))
                const_loads.append(nc.sync.dma_start(
                    out=wq_sb[:, ck * QD:(ck + 1) * QD],
                    in_=wqT[ck * 128:(ck + 1) * 128, :]))
            for j in range(4):
                const_loads.append(nc.sync.dma_start(
                    out=wo_sb[:, j * D:(j + 1) * D],
                    in_=woT[j * 128:(j + 1) * 128, :]))
            const_loads.append(nc.sync.dma_start(out=bq_sb[:, :], in_=bq4[:, :]))
            const_loads.append(nc.sync.dma_start(out=bk_sb[:, :], in_=bkT[:, :]))
            const_loads.append(nc.sync.dma_start(out=bv_sb[:, :], in_=bvT[:, :]))
            const_loads.append(nc.sync.dma_start(out=eye_sb[:, :], in_=eye[:, :]))
            # ones columns of vp0/vp1 and ones65: memset whole tiles to 1.0;
            # V-proj writes only the 64 data columns of each 65-chunk.
            const_loads.append(nc.vector.memset(vp0[:, :], 1.0))
            const_loads.append(nc.vector.memset(vp1[:, :], 1.0))
            const_loads.append(nc.vector.memset(ones65[:, :], 1.0))
            _pe_touch(nc, const_loads)

            # ---------------- projection phase ----------------
            proj_tail = []
            with tc.tile_pool(name="proj", bufs=1) as jp, \
                 tc.tile_pool(name="projps", bufs=1, space="PSUM") as jpp:
                # K projection (needed first: attention needs all of kT)
                for tt in range(NTT):
                    ps = jpp.tile([128, TT], F32, tag="kp", bufs=2)
                    for ck in range(NCK):
                        nc.tensor.matmul(
                            ps[:, :],
                            wk_sb[:, ck * 128:(ck + 1) * 128],
                            xT_sb[:, ck * TOKS + tt * TT:
                                  ck * TOKS + (tt + 1) * TT],
                            start=(ck == 0), stop=(ck == NCK - 1))
                    nc.vector.tensor_scalar_add(
                        kT[:, tt * TT:(tt + 1) * TT], ps[:, :], bk_sb[:, 0:1])
                # vT [v-dims, keys] via 512-row matmuls (bias varies along
                # partitions -> tensor_scalar_add)
                for tt in range(NTT):
                    ps = jpp.tile([128, TT], F32, tag="vt", bufs=2)
                    for ck in range(NCK):
                        nc.tensor.matmul(
                            ps[:, :],
                            wv_sb[:, ck * 128:(ck + 1) * 128],
                            xT_sb[:, ck * TOKS + tt * TT:
                                  ck * TOKS + (tt + 1) * TT],
                            start=(ck == 0), stop=(ck == NCK - 1))
                    nc.vector.tensor_scalar_add(
                        vT_sb[:, tt * TT:(tt + 1) * TT], ps[:, :],
                        bv_sb[:, 0:1])
                # Q projection: 4 chunks (head pairs) per token tile;
                # V' transposes interleaved after the first Q tile so their
                # DVE-produced input is ready by the time PE reaches them
                for tt in range(NTT):
                    for j in range(4):
                        ps = jpp.tile([128, TT], F32, tag="qp", bufs=2)
                        for ck in range(NCK):
                            nc.tensor.matmul(
                                ps[:, :],
                                wq_sb[:, ck * QD + j * 128:
                                      ck * QD + (j + 1) * 128],
                                xT_sb[:, ck * TOKS + tt * TT:
                                      ck * TOKS + (tt + 1) * TT],
                                start=(ck == 0), stop=(ck == NCK - 1))
                        nc.vector.tensor_scalar_add(
                            qT[:, j * TOKS + tt * TT: j * TOKS + (tt + 1) * TT],
                            ps[:, :], bq_sb[:, j:j + 1])
                    if tt == 0:
                        # V' natural layout [keys, v-dims] via PE transpose
                        for kc in range(NKC):
                            tp = jpp.tile([128, 128], BF16, tag="tp", bufs=2)
                            nc.tensor.transpose(
                                tp[:, :], vT_sb[:, kc * 128:(kc + 1) * 128],
                                eye_sb[:, :])
                            a0 = nc.vector.tensor_copy(
                                vp0[:, kc * V65: kc * V65 + DH], tp[:, 0:DH])
                            a1 = nc.vector.tensor_copy(
                                vp1[:, kc * V65: kc * V65 + DH],
                                tp[:, DH:128])
                            if kc == NKC - 1:
                                proj_tail += [a0, a1]

            _pe_touch(nc, proj_tail)

            # ---------------- attention + out-projection ----------------
            with tc.tile_pool(name="attn", bufs=1) as ap, \
                 tc.tile_pool(name="attnps", bufs=1, space="PSUM") as app:

                def op_group(qt, n, m, oT_t):
                    # out[qt*512+m*128 : +128, n*512 : +512] partial
                    op = app.tile([128, TT], F32, tag="op", bufs=1)
                    for j in range(4):
                        nc.tensor.matmul(
                            op[:, :],
                            oT_t[:, j * QTILE + m * 128:
                                 j * QTILE + (m + 1) * 128],
                            wo_sb[:, j * D + n * 512: j * D + (n + 1) * 512],
                            start=(j == 0), stop=(j == 3))
                    osb = ap.tile([128, 512], F32, tag="osb", bufs=4)
                    nc.vector.tensor_copy(osb[:, :], op[:, :])
                    nc.sync.dma_start(
                        out=out[qt * QTILE + m * 128:
                                qt * QTILE + (m + 1) * 128,
                                n * 512:(n + 1) * 512],
                        in_=osb[:, :])

                def spair(qt, j, kc):
                    Sp = app.tile([128, 2 * QTILE], F32, tag="S", bufs=2)
                    qs = j * TOKS + qt * QTILE
                    nc.tensor.matmul(
                        Sp[:, 0:QTILE],
                        kT[0:64, kc * 128:(kc + 1) * 128],
                        qT[0:64, qs:qs + QTILE],
                        start=True, stop=True)
                    nc.tensor.matmul(
                        Sp[:, QTILE:2 * QTILE],
                        kT[64:128, kc * 128:(kc + 1) * 128],
                        qT[64:128, qs:qs + QTILE],
                        start=True, stop=True)
                    return Sp

                def normalize(opj0, opj1, oT_t, j):
                    # oT = num * (1/den); den broadcast along partitions
                    # via PE.  Numerators and denominators are staged out
                    # of PSUM first (cheap copies) so the opj banks free up
                    # for the next segment's first AV before the expensive
                    # multi-pass reciprocal runs; half-1 is partition-
                    # shifted 0:64 -> 64:128 by an SBUF->SBUF DMA.
                    stgA = ap.tile([64, 2 * QTILE], BF16, tag="stgA", bufs=2)
                    nc.vector.tensor_copy(stgA[:, 0:QTILE], opj0[0:64, :])
                    nc.vector.tensor_copy(stgA[:, QTILE:2 * QTILE],
                                          opj1[0:64, :])
                    dsb = ap.tile([65, 2 * QTILE], F32, tag="dsb", bufs=2)
                    nc.vector.tensor_copy(dsb[64:65, 0:QTILE],
                                          opj0[64:65, :])
                    nc.vector.tensor_copy(dsb[64:65, QTILE:2 * QTILE],
                                          opj1[64:65, :])
                    rf = ap.tile([65, 2 * QTILE], F32, tag="rf", bufs=2)
                    nc.vector.reciprocal(rf[64:65, :], dsb[64:65, :])
                    rb = ap.tile([65, 2 * QTILE], BF16, tag="rb", bufs=2)
                    nc.vector.tensor_copy(rb[64:65, :], rf[64:65, :])
                    stgB = ap.tile([128, QTILE], BF16, tag="stgB", bufs=2)
                    nc.sync.dma_start(out=stgB[64:128, :],
                                      in_=stgA[0:64, QTILE:2 * QTILE])
                    bcj = app.tile([128, QTILE], F32, tag="bcj", bufs=1)
                    nc.tensor.matmul(
                        bcj[0:64, :], ones65[64:65, :],
                        rb[64:65, 0:QTILE], start=True, stop=True)
                    nc.tensor.matmul(
                        bcj[64:128, :], ones65[64:65, :],
                        rb[64:65, QTILE:2 * QTILE], start=True, stop=True)
                    bcs = ap.tile([128, QTILE], BF16, tag="bcs", bufs=2)
                    nc.vector.tensor_copy(bcs[:, :], bcj[:, :])
                    nc.vector.tensor_mul(
                        oT_t[0:64, j * QTILE:(j + 1) * QTILE],
                        stgA[0:64, 0:QTILE], bcs[0:64, :])
                    nc.vector.tensor_mul(
                        oT_t[64:128, j * QTILE:(j + 1) * QTILE],
                        stgB[64:128, :], bcs[64:128, :])

                # software-pipelined emission: per round, ACT gets exp(k)
                # first, then PE gets S-pair(k+1) (independent of exp(k)),
                # an out-proj filler group, and only then AV-pair(k).
                rounds = [(qt, j, kc) for qt in range(NQT)
                          for j in range(4) for kc in range(NKC)]
                pending, prev_ops, pi = [], [], 0
                oT_sb = None
                opj0 = opj1 = None
                Sp_cur = spair(*rounds[0])
                for i, (qt, j, kc) in enumerate(rounds):
                    if j == 0 and kc == 0:
                        oT_sb = ap.tile([128, 4 * QTILE], BF16, tag="oTsb",
                                        bufs=2)
                        prev_ops, pi = pending, 0
                        pending = [(qt, n, m, oT_sb)
                                   for n in range(4) for m in range(4)]
                    if kc == 0:
                        opj0 = app.tile([65, QTILE], F32, tag="opj0", bufs=1)
                        opj1 = app.tile([65, QTILE], F32, tag="opj1", bufs=1)
                    E = ap.tile([128, 2 * QTILE], BF16, tag="E", bufs=4)
                    nc.scalar.activation(
                        E[:, :], Sp_cur[:, :], AF.Exp, scale=SCALE)
                    Sp_nxt = (spair(*rounds[i + 1])
                              if i + 1 < len(rounds) else None)
                    if kc % 4 == 0 and pi < len(prev_ops):
                        op_group(*prev_ops[pi])
                        pi += 1
                    # AV: stationary [v-dims | ones] -> rows 0:65
                    # (row 64 = denominator)
                    nc.tensor.matmul(
                        opj0[0:V65, :],
                        vp0[:, kc * V65:(kc + 1) * V65],
                        E[:, 0:QTILE],
                        start=(kc == 0), stop=(kc == NKC - 1))
                    nc.tensor.matmul(
                        opj1[0:V65, :],
                        vp1[:, kc * V65:(kc + 1) * V65],
                        E[:, QTILE:2 * QTILE],
                        start=(kc == 0), stop=(kc == NKC - 1))
                    if kc == NKC - 1:
                        normalize(opj0, opj1, oT_sb, j)
                    Sp_cur = Sp_nxt
                while pi < len(prev_ops):
                    op_group(*prev_ops[pi])
                    pi += 1
                for args in pending:
                    op_group(*args)
    return nc


def _prep_inputs(x, Wq, bq, Wk, bk, Wv, bv, Wo, bo):
    """Build the 8 per-core input maps."""
    f = np.float32
    bf = ml_dtypes.bfloat16
    x = np.asarray(x, f)
    Wq, bq = np.asarray(Wq, f), np.asarray(bq, f)
    Wk, bk = np.asarray(Wk, f), np.asarray(bk, f)
    Wv, bv = np.asarray(Wv, f), np.asarray(bv, f)
    Wo = np.asarray(Wo, f)
    # per-core head-dim permutation within the group's 512 q dims
    perm = np.concatenate([
        np.arange(h * DH, (h + 1) * DH) for h in HEAD_ORDER])
    eye = np.eye(128, dtype=f).astype(bf)
    in_maps = []
    for c in range(8):
        b, g = divmod(c, 4)
        wq_g = Wq[g * QD:(g + 1) * QD, :][perm, :]     # (512, 2048)
        bq_g = bq[g * QD:(g + 1) * QD][perm]
        wo_g = Wo[:, g * QD:(g + 1) * QD].T[perm, :]   # (512, 2048)
        in_maps.append({
            "xT": np.ascontiguousarray(x[b].T.astype(bf)),
            "wqT": np.ascontiguousarray(wq_g.T.astype(bf)),
            "wkT": np.ascontiguousarray(Wk[g * 128:(g + 1) * 128, :].T
                                        .astype(bf)),
            "wvT": np.ascontiguousarray(Wv[g * 128:(g + 1) * 128, :].T
                                        .astype(bf)),
            "woT": np.ascontiguousarray(wo_g.astype(bf)),
            "bq4": np.ascontiguousarray(bq_g.reshape(4, 128).T),
            "bkT": np.ascontiguousarray(bk[g * 128:(g + 1) * 128, None]),
            "bvT": np.ascontiguousarray(bv[g * 128:(g + 1) * 128, None]),
            "eye": eye,
        })
    return in_maps


def run(inputs, trace=False, **kw):
    nc = _build()
    _orig_tjb = nc.to_json_bytes
    nc.to_json_bytes = lambda: _split_matmul_waits(_orig_tjb())
    in_maps = _prep_inputs(**inputs)
    res = run_bass_kernel_spmd(nc, in_maps, list(range(8)), trace=trace, **kw)
    bo = np.asarray(inputs["bo"], np.float32)
    y = np.empty((2, TOKS, D), np.float32)
    for b in range(2):
        acc = res.results[4 * b]["out"].astype(np.float32)
        for g in range(1, 4):
            acc = acc + res.results[4 * b + g]["out"]
        y[b] = acc + bo[None, :]
    return y, res


def kernel(**inputs):
    y, _ = run(inputs, trace=False)
    return y
